# revision 11
# baseline (speedup 1.0000x reference)
"""Multi-head self-attention (RoPE, causal) Trainium2 Bass kernel.

Sharding: tensor-parallel over heads. 16 heads / 8 cores = 2 heads per core.
Each core computes Q/K/V projections for its 2 heads, causal flash attention,
and a partial output projection against its 256-column slice of Wo. The host
sums the 8 partial [S, D] outputs.

All matmuls run in bf16 with fp32 PSUM accumulation. Softmax skips the max
subtraction (scores are O(1) for this problem family; exp stays in fp32
range), so the denominator comes free via a ones-column appended to V.
RoPE's even/odd interleave is folded into a row permutation of Wq/Wk, making
the on-device rotation a contiguous rotate-half.
"""
import sys

sys.path.insert(0, "/opt/trn_rl_repo")

import numpy as np
import ml_dtypes

import concourse.bass as bass  # noqa: F401  (registers AP machinery)
import concourse.tile as tile
from concourse import bacc, mybir
from concourse import bass_utils
from concourse.masks import make_identity

BF16 = ml_dtypes.bfloat16
S = 4096
D = 2048
DH = 128
N_CORES = 8
HPC = 2  # heads per core
PW = 512  # projection s-window
QW = 1024  # attention q-window
N_QW = S // QW  # 4
SUBS = QW // 128  # 8 q-subtiles per window
N_KT = S // 128  # 32 k-tiles
INV_SQRT_DH = float(1.0 / np.sqrt(128.0))

_CACHE = {}


def _build(dbg=False):
    fp32 = mybir.dt.float32
    bf16 = mybir.dt.bfloat16

    nc = bacc.Bacc("TRN2", target_bir_lowering=False, debug=False,
                   num_devices=N_CORES)
    xT_d = nc.dram_tensor("xT", [D, S], bf16, kind="ExternalInput").ap()
    wq_d = nc.dram_tensor("wqT", [D, HPC * DH], bf16, kind="ExternalInput").ap()
    wk_d = nc.dram_tensor("wkT", [D, HPC * DH], bf16, kind="ExternalInput").ap()
    wv_d = nc.dram_tensor("wvT", [D, HPC * DH], bf16, kind="ExternalInput").ap()
    wo_d = nc.dram_tensor("woT", [HPC * DH, D], bf16, kind="ExternalInput").ap()
    cos_d = nc.dram_tensor("cosF", [128, S], bf16, kind="ExternalInput").ap()
    sin_d = nc.dram_tensor("sinX", [128, S], bf16, kind="ExternalInput").ap()
    swp_d = nc.dram_tensor("swp", [128, 128], bf16, kind="ExternalInput").ap()
    mask_d = nc.dram_tensor("mask", [128, 128], bf16, kind="ExternalInput").ap()
    out_d = nc.dram_tensor("out", [S, D], fp32, kind="ExternalOutput").ap()
    if dbg:
        bf = mybir.dt.bfloat16
        dbg_qt = nc.dram_tensor("dbg_qt", [128, S], bf, kind="ExternalOutput").ap()
        dbg_kt = nc.dram_tensor("dbg_kt", [128, S], bf, kind="ExternalOutput").ap()
        dbg_v = nc.dram_tensor("dbg_v", [128, 2 * (DH + 1)], bf,
                               kind="ExternalOutput").ap()
        dbg_oc = nc.dram_tensor("dbg_oc", [128, S], bf, kind="ExternalOutput").ap()

    xT_r = xT_d.rearrange("(t p) s -> p t s", p=128)   # [128, 16, S]
    wq_r = wq_d.rearrange("(t p) m -> p t m", p=128)   # [128, 16, 256]
    wk_r = wk_d.rearrange("(t p) m -> p t m", p=128)
    wv_r = wv_d.rearrange("(t p) m -> p t m", p=128)

    EXP = mybir.ActivationFunctionType.Exp

    with tile.TileContext(nc) as tc:
        with tc.tile_pool(name="persist", bufs=1) as pp:
            wq_sb = pp.tile([128, 16, HPC * DH], bf16, tag="wq")
            wk_sb = pp.tile([128, 16, HPC * DH], bf16, tag="wk")
            wv_sb = pp.tile([128, 16, HPC * DH], bf16, tag="wv")
            nc.sync.dma_start(out=wq_sb, in_=wq_r)
            nc.sync.dma_start(out=wk_sb, in_=wk_r)
            nc.sync.dma_start(out=wv_sb, in_=wv_r)
            wo_sb = []
            for t in range(HPC):
                w = pp.tile([128, D], bf16, tag=f"wo{t}", name=f"wo{t}")
                nc.sync.dma_start(out=w, in_=wo_d[t * 128:(t + 1) * 128, :])
                wo_sb.append(w)
            cos_sb = pp.tile([128, S], bf16, tag="cos")
            sin_sb = pp.tile([128, S], bf16, tag="sin")
            nc.sync.dma_start(out=cos_sb, in_=cos_d)
            nc.sync.dma_start(out=sin_sb, in_=sin_d)
            mask_sb = pp.tile([128, 128], bf16, tag="mask")
            nc.sync.dma_start(out=mask_sb, in_=mask_d)
            swp_sb = pp.tile([128, 128], bf16, tag="swp")
            nc.sync.dma_start(out=swp_sb, in_=swp_d)
            ident_sb = pp.tile([128, 128], bf16, tag="ident")
            make_identity(nc, ident_sb)

            qt = [pp.tile([128, S], bf16, tag=f"qt{h}", name=f"qt{h}") for h in range(HPC)]
            kt = [pp.tile([128, S], bf16, tag=f"kt{h}", name=f"kt{h}") for h in range(HPC)]
            v_sb = [pp.tile([128, 2 * (DH + 1)], bf16, tag=f"v{i}", name=f"v{i}")
                    for i in range(N_KT)]
            oc = [pp.tile([128, S], bf16, tag=f"oc{h}", name=f"oc{h}") for h in range(HPC)]

            # ---------------- Phase A: projections + RoPE ----------------
            with tc.tile_pool(name="xw", bufs=2) as xwp, \
                 tc.tile_pool(name="ropet", bufs=2) as rtp, \
                 tc.tile_pool(name="psqk", bufs=3, space="PSUM") as psqk, \
                 tc.tile_pool(name="psv", bufs=2, space="PSUM") as psv:
                for w in range(S // PW):
                    sl = slice(w * PW, (w + 1) * PW)
                    xw = xwp.tile([128, 16, PW], bf16, tag="xw")
                    nc.sync.dma_start(out=xw, in_=xT_r[:, :, sl])
                    for h in range(HPC):
                        hs = slice(h * DH, (h + 1) * DH)
                        for wsb, dest in ((wq_sb, qt[h]), (wk_sb, kt[h])):
                            ps = psqk.tile([128, PW], fp32, tag="qk")
                            for t in range(16):
                                nc.tensor.matmul(ps, wsb[:, t, hs], xw[:, t, :],
                                                 start=(t == 0), stop=(t == 15))
                            nc.scalar.copy(out=dest[:, sl], in_=ps)
                            # rope in place on this window:
                            # dest = dest*cosF + swap_halves(dest)*[-sin;sin]
                            dsl = dest[:, sl]
                            swp = psqk.tile([128, PW], fp32, tag="swp",
                                            bufs=2)
                            nc.tensor.matmul(swp, swp_sb, dsl,
                                             start=True, stop=True)
                            m1 = rtp.tile([128, PW], bf16, tag="m1")
                            m2 = rtp.tile([128, PW], bf16, tag="m2")
                            nc.vector.tensor_mul(m1, dsl, cos_sb[:, sl])
                            nc.vector.tensor_mul(m2, swp, sin_sb[:, sl])
                            nc.vector.tensor_add(dsl, m1, m2)
                    for sub in range(PW // 128):
                        st = w * (PW // 128) + sub
                        ssl = slice(sub * 128, (sub + 1) * 128)
                        pv = psv.tile([128, HPC * DH], fp32, tag="v")
                        for t in range(16):
                            nc.tensor.matmul(pv, xw[:, t, ssl], wv_sb[:, t, :],
                                             start=(t == 0), stop=(t == 15))
                        vt = v_sb[st]
                        nc.vector.memset(vt[:, DH:DH + 1], 1.0)
                        nc.vector.memset(vt[:, 2 * DH + 1:2 * DH + 2], 1.0)
                        nc.scalar.copy(out=vt[:, 0:DH], in_=pv[:, 0:DH])
                        nc.scalar.copy(out=vt[:, DH + 1:2 * DH + 1],
                                       in_=pv[:, DH:2 * DH])

            if dbg:
                nc.sync.dma_start(out=dbg_qt, in_=qt[0])
                nc.sync.dma_start(out=dbg_kt, in_=kt[0])
                nc.sync.dma_start(out=dbg_v, in_=v_sb[1])

            # ---------------- Phase B: causal attention ----------------
            # pt_j strips for a whole q-window stay in SBUF; each q-subtile's
            # PV accumulation owns a full PSUM bank (start=True invalidates
            # has_written for the whole bank, so packing accumulators into
            # one bank is unsafe).
            with tc.tile_pool(name="pt", bufs=1) as ptp, \
                 tc.tile_pool(name="bst", bufs=4) as bst, \
                 tc.tile_pool(name="pssc", bufs=2, space="PSUM") as pssc, \
                 tc.tile_pool(name="psaug", bufs=3, space="PSUM") as psaug, \
                 tc.tile_pool(name="pstr", bufs=1, space="PSUM") as pstr:
                for h in range(HPC):
                    vsl = slice(h * (DH + 1), (h + 1) * (DH + 1))
                    for w in range(N_QW):
                        q0 = w * QW
                        n_j = SUBS * w + SUBS
                        pts = []
                        for j in range(n_j):
                            ksl = slice(j * 128, (j + 1) * 128)
                            sc = pssc.tile([128, QW], fp32, tag="sc")
                            nc.tensor.matmul(sc[:, 0:512], kt[h][:, ksl],
                                             qt[h][:, q0:q0 + 512],
                                             start=True, stop=True)
                            nc.tensor.matmul(sc[:, 512:1024], kt[h][:, ksl],
                                             qt[h][:, q0 + 512:q0 + 1024],
                                             start=True, stop=True)
                            pt = ptp.tile([128, QW], bf16, tag=f"pt{j}",
                                          name=f"pt{j}")
                            nc.scalar.activation(pt, sc, EXP, scale=INV_SQRT_DH)
                            if j >= SUBS * w:
                                c = j - SUBS * w
                                csl = slice(c * 128, (c + 1) * 128)
                                nc.vector.tensor_mul(pt[:, csl], pt[:, csl],
                                                     mask_sb)
                            pts.append(pt)
                        for il in range(SUBS):
                            i = SUBS * w + il
                            isl = slice(il * 128, (il + 1) * 128)
                            aug = psaug.tile([128, DH + 1], fp32, tag="aug")
                            for j in range(i + 1):
                                nc.tensor.matmul(aug, pts[j][:, isl],
                                                 v_sb[j][:, vsl],
                                                 start=(j == 0), stop=(j == i))
                            rc = bst.tile([128, 1], fp32, tag="rc")
                            nc.vector.reciprocal(rc, aug[:, DH:DH + 1])
                            stg = bst.tile([128, 128], bf16, tag="st")
                            nc.vector.tensor_scalar_mul(stg, aug[:, 0:DH], rc)
                            tr = pstr.tile([128, 128], bf16, tag="tr")
                            nc.tensor.transpose(tr, stg, ident_sb)
                            nc.scalar.copy(out=oc[h][:, i * 128:(i + 1) * 128],
                                           in_=tr)

            if dbg:
                nc.sync.dma_start(out=dbg_oc, in_=oc[0])

            # ---------------- Phase C: output projection ----------------
            with tc.tile_pool(name="cst", bufs=4) as cst, \
                 tc.tile_pool(name="psc", bufs=4, space="PSUM") as psc:
                for m in range(S // 128):
                    msl = slice(m * 128, (m + 1) * 128)
                    for nw in range(D // 512):
                        nsl = slice(nw * 512, (nw + 1) * 512)
                        ps = psc.tile([128, 512], fp32, tag="c")
                        for t in range(HPC):
                            nc.tensor.matmul(ps, oc[t][:, msl], wo_sb[t][:, nsl],
                                             start=(t == 0), stop=(t == HPC - 1))
                        so = cst.tile([128, 512], fp32, tag="so")
                        if (m + nw) % 2 == 0:
                            nc.vector.tensor_copy(so, ps)
                        else:
                            nc.scalar.copy(out=so, in_=ps)
                        nc.sync.dma_start(out=out_d[msl, nsl], in_=so)

    nc.compile()
    return nc


def _host_prep(inputs):
    x = np.ascontiguousarray(np.asarray(inputs["x"], dtype=np.float32)[0])  # [S, D]
    tp = np.asarray(inputs["token_positions"]).reshape(-1)[:S]
    Wq = np.asarray(inputs["Wq"], dtype=np.float32)
    Wk = np.asarray(inputs["Wk"], dtype=np.float32)
    Wv = np.asarray(inputs["Wv"], dtype=np.float32)
    Wo = np.asarray(inputs["Wo"], dtype=np.float32)

    xT = np.ascontiguousarray(x.T).astype(BF16)  # [D, S]

    # f32 RoPE tables, replicated across the two 64-row halves
    inv_freq = (10000.0 ** (-np.arange(0, DH, 2, dtype=np.float32) / DH)
                ).astype(np.float32)
    ang = tp.astype(np.float32)[:, None] * inv_freq[None, :]  # [S, 64] f32
    cos = np.cos(ang).astype(np.float32).T  # [64, S]
    sin = np.sin(ang).astype(np.float32).T
    cosF = np.concatenate([cos, cos], axis=0).astype(BF16)  # [128, S]
    sinX = np.concatenate([-sin, sin], axis=0).astype(BF16)
    # half-swap permutation as a matmul lhsT: out[m] = in[(m+64) % 128]
    swp = np.zeros((128, 128), dtype=np.float32)
    swp[np.arange(128), (np.arange(128) + 64) % 128] = 1.0
    swp = swp.astype(BF16)

    # causal mask in scores^T layout: valid iff k <= q  ->  upper triangular
    mask = np.triu(np.ones((128, 128), dtype=np.float32)).astype(BF16)

    perm = np.concatenate([np.arange(0, DH, 2), np.arange(1, DH, 2)])
    in_maps = []
    for c in range(N_CORES):
        rows = slice(c * HPC * DH, (c + 1) * HPC * DH)
        wq_blk = Wq[rows].reshape(HPC, DH, D)[:, perm].reshape(HPC * DH, D)
        wk_blk = Wk[rows].reshape(HPC, DH, D)[:, perm].reshape(HPC * DH, D)
        wv_blk = Wv[rows]
        in_maps.append({
            "xT": xT,
            "wqT": np.ascontiguousarray(wq_blk.T).astype(BF16),
            "wkT": np.ascontiguousarray(wk_blk.T).astype(BF16),
            "wvT": np.ascontiguousarray(wv_blk.T).astype(BF16),
            "woT": np.ascontiguousarray(Wo[:, rows].T).astype(BF16),
            "cosF": cosF,
            "sinX": sinX,
            "swp": swp,
            "mask": mask,
        })
    return in_maps


def get_compiled():
    if "nc" not in _CACHE:
        _CACHE["nc"] = _build()
    return _CACHE["nc"]


def kernel(**inputs):
    nc = get_compiled()
    in_maps = _host_prep(inputs)
    res = bass_utils.run_bass_kernel_spmd(
        nc, in_maps, core_ids=list(range(N_CORES)))
    y = np.zeros((S, D), dtype=np.float32)
    for c in range(N_CORES):
        y += res.results[c]["out"]
    return y.reshape(1, S, D)
